# revision 8
# baseline (speedup 1.0000x reference)
"""Chunked delta-rule Trainium2 kernel (C=8 steps per chunk).

Algebra (reversed steps g, u0 = q):
  d_g = k_g.u_g ; y += d_g k_g ; u += d_g kt_g   (kt = -alpha k)
With d' := -alpha d, per chunk n of C steps:
  c_n = K_n u_n                        (u at chunk entry)
  d'_n = X_n c_n,  X_n = M^{-1},  M = diag(-1/alpha) - G_low  (per lane)
  u_{n+1} = u_n + K_n^T d'_n
  y += K_n^T (d'_n * (-1/alpha_i))
  carry: c_{n+1} = K_{n+1} u_n + Gcr_n d'_n   (Gcr = cross-chunk raw gram)
X_n computed on device by batched forward substitution from the
host-gathered band Abar[i,j] = -alpha_i G[t_i,t_j] (pure table gathers).
All k-streams bf16, all reductions/accumulators f32 (validated 2.4e-3).
"""

import numpy as np

B, L, H, V = 1024, 2048, 32, 64
N_CORES = 8
BL = B // N_CORES
T = L - 1
C = 8                       # steps per chunk
NCH = (T + C - 1) // C      # 256 chunks
TP = NCH * C
SLAB = 16                   # chunks per DMA slab
NSL = NCH // SLAB           # 16 slabs
YF = 16                     # chunks per y-fold
LN_EPS = 1e-5
DELTA_EPS = 1e-6

_BUILT = {}


def _build_module():
    import concourse.bass as bass  # noqa: F401
    import concourse.mybir as mybir
    import concourse.tile as tile
    from concourse import bacc
    from concourse.masks import make_identity

    f32 = mybir.dt.float32
    bf16 = mybir.dt.bfloat16
    OP = mybir.AluOpType
    AX = mybir.AxisListType

    nc = bacc.Bacc("TRN2", target_bir_lowering=False, debug=False,
                   num_devices=N_CORES)

    ktI = nc.dram_tensor("ktI", [BL, NCH, C, H], bf16, kind="ExternalInput")
    ktH = nc.dram_tensor("ktH", [BL, NCH, H, 2, C], bf16,
                         kind="ExternalInput")
    abar = nc.dram_tensor("abar", [BL, NCH, C, C], bf16, kind="ExternalInput")
    x0 = nc.dram_tensor("x0", [BL, NCH, C, C], bf16, kind="ExternalInput")
    gcr = nc.dram_tensor("gcr", [BL, NCH, C, C], bf16, kind="ExternalInput")
    qin = nc.dram_tensor("qin", [BL, H], f32, kind="ExternalInput")
    rw = nc.dram_tensor("rw", [H, H], f32, kind="ExternalInput")
    rb = nc.dram_tensor("rb", [H, 1], f32, kind="ExternalInput")
    ow = nc.dram_tensor("ow", [H, V], f32, kind="ExternalInput")
    ob = nc.dram_tensor("ob", [V, 1], f32, kind="ExternalInput")
    outT = nc.dram_tensor("outT", [V, BL], f32, kind="ExternalOutput")

    with tile.TileContext(nc) as tc, nc.allow_low_precision("bf16 scan"):
        with (
            tc.tile_pool(name="persist", bufs=1) as persist,
            tc.tile_pool(name="kIp", bufs=2) as kIp,
            tc.tile_pool(name="kHp", bufs=2) as kHp,
            tc.tile_pool(name="abp", bufs=2) as abp,
            tc.tile_pool(name="gcp", bufs=2) as gcp,
            tc.tile_pool(name="work", bufs=2) as work,
            tc.tile_pool(name="dp", bufs=3) as dp,
            tc.tile_pool(name="yp", bufs=2) as yp,
            tc.tile_pool(name="psum_r", bufs=1, space="PSUM") as psum_r,
        ):
            uq = persist.tile([BL, H], f32)
            nc.sync.dma_start(uq[:], qin.ap())
            ubf = persist.tile([BL, H], bf16)
            nc.vector.tensor_copy(out=ubf[:], in_=uq[:])
            y = persist.tile([BL, H], f32)
            nc.vector.memset(y[:], 0.0)
            rw_sb = persist.tile([H, H], f32)
            nc.sync.dma_start(rw_sb[:], rw.ap())
            rb_sb = persist.tile([H, 1], f32)
            nc.sync.dma_start(rb_sb[:], rb.ap())
            ow_sb = persist.tile([H, V], f32)
            nc.sync.dma_start(ow_sb[:], ow.ap())
            ob_sb = persist.tile([V, 1], f32)
            nc.sync.dma_start(ob_sb[:], ob.ap())
            ident = persist.tile([BL, BL], f32)
            make_identity(nc, ident[:])

            # X for ALL chunks, layout XT[b, n, e, j] = X[row j, col e]
            XT = persist.tile([BL, NCH, C, C], bf16)
            # y slots: [BL, H, YF, 2C]; y half = [:, :, :, C:2C]
            yms = persist.tile([BL, H, YF, 2, C], bf16)

            def dma_slab(s):
                n0 = s * SLAB
                kI = kIp.tile([BL, SLAB, C, H], bf16, tag="kI")
                nc.sync.dma_start(kI[:], ktI.ap()[:, n0:n0 + SLAB])
                kH = kHp.tile([BL, SLAB, H, 2, C], bf16, tag="kH")
                nc.sync.dma_start(kH[:], ktH.ap()[:, n0:n0 + SLAB])
                ab = abp.tile([BL, SLAB, C, C], bf16, tag="ab")
                nc.sync.dma_start(ab[:], abar.ap()[:, n0:n0 + SLAB])
                nc.sync.dma_start(XT[:, n0:n0 + SLAB], x0.ap()[:, n0:n0 + SLAB])
                gc = gcp.tile([BL, SLAB, C, C], bf16, tag="gc")
                nc.sync.dma_start(gc[:], gcr.ap()[:, n0:n0 + SLAB])
                return kI, kH, ab, gc

            def subst_ops(s, ab):
                """Yield the 16 substitution micro-ops for slab s."""
                n0 = s * SLAB
                tmp = work.tile([BL, SLAB, C, C], bf16, tag="tmp")
                for i in range(1, C):
                    yield lambda i=i: nc.vector.tensor_tensor(
                        out=tmp[:, :, 0:i, 0:i],
                        in0=XT[:, n0:n0 + SLAB, 0:i, 0:i],
                        in1=ab[:, :, i:i + 1, 0:i].to_broadcast(
                            [BL, SLAB, i, i]),
                        op=OP.mult)
                    yield lambda i=i: nc.vector.tensor_reduce(
                        out=XT[:, n0:n0 + SLAB, 0:i, i:i + 1].rearrange(
                            "p n e j -> p n (e j)"),
                        in_=tmp[:, :, 0:i, 0:i],
                        axis=AX.X, op=OP.add)

            # prologue: slab 0 DMA + substitution up-front
            cur = dma_slab(0)
            for op in subst_ops(0, cur[2]):
                op()
            nxt = None
            nxt_gen = None
            dzero = persist.tile([BL, C], bf16)
            nc.vector.memset(dzero[:], 0.0)
            dprev = dzero
            # P tile for chunk 0: carry rows zeroed, cbase in col C
            Pcur = dp.tile([BL, C, C + 2], bf16, tag="P")
            nc.vector.memset(Pcur[:], 0.0)
            cb0 = work.tile([BL, C, H], bf16, tag="cb")
            nc.vector.tensor_tensor(
                out=cb0[:], in0=cur[0][:, 0],
                in1=ubf[:].rearrange("p (o h) -> p o h", o=1)
                .to_broadcast([BL, C, H]),
                op=OP.mult)
            nc.vector.tensor_reduce(
                out=Pcur[:, :, C:C + 1].rearrange("p c o -> p (c o)"),
                in_=cb0[:], axis=AX.X, op=OP.add)

            for s in range(NSL):
                if s + 1 < NSL:
                    nxt = dma_slab(s + 1)
                    nxt_gen = subst_ops(s + 1, nxt[2])
                else:
                    nxt_gen = None
                for m in range(SLAB):
                    n = s * SLAB + m
                    kI, kH, ab, gc = cur
                    # ---- chain: m1 (carry), r1, m2, r2 ----
                    nc.vector.tensor_tensor(
                        out=Pcur[:, :, 0:C], in0=gc[:, m],
                        in1=dprev[:].rearrange("p (o c) -> p o c", o=1)
                        .to_broadcast([BL, C, C]),
                        op=OP.mult)
                    cbf = dp.tile([BL, C], bf16, tag="cbf")
                    nc.vector.tensor_reduce(
                        out=cbf[:], in_=Pcur[:, :, 0:C + 1], axis=AX.X,
                        op=OP.add)
                    Q = dp.tile([BL, C, C], bf16, tag="Q")
                    xv = XT[:, n].rearrange("p e j -> p j e")
                    nc.vector.tensor_tensor(
                        out=Q[:], in0=xv,
                        in1=cbf[:].rearrange("p (o c) -> p o c", o=1)
                        .to_broadcast([BL, C, C]),
                        op=OP.mult)
                    dbf = dp.tile([BL, C], bf16, tag="dbf")
                    nc.vector.tensor_reduce(
                        out=dbf[:], in_=Q[:], axis=AX.X, op=OP.add)
                    # ---- off-chain: cbase(n+1) against u_n (ubf not yet
                    # updated), then u update, y mult, fillers ----
                    if n + 1 < NCH:
                        if m + 1 < SLAB:
                            kI2, m2i = kI, m + 1
                        else:
                            kI2, m2i = nxt[0], 0
                        Pnxt = dp.tile([BL, C, C + 2], bf16, tag="P")
                        cb = work.tile([BL, C, H], bf16, tag="cb")
                        nc.vector.tensor_tensor(
                            out=cb[:], in0=kI2[:, m2i],
                            in1=ubf[:].rearrange("p (o h) -> p o h", o=1)
                            .to_broadcast([BL, C, H]),
                            op=OP.mult)
                        nc.vector.tensor_reduce(
                            out=Pnxt[:, :, C:C + 1].rearrange(
                                "p c o -> p (c o)"),
                            in_=cb[:], axis=AX.X, op=OP.add)
                    else:
                        Pnxt = None
                    # u-half mult + u update (completes u_{n+1})
                    slot = yms[:, :, n % YF]
                    nc.vector.tensor_tensor(
                        out=slot[:, :, 0],
                        in0=kH[:, m, :, 0],
                        in1=dbf[:].rearrange("p (o c) -> p o c", o=1)
                        .to_broadcast([BL, H, C]),
                        op=OP.mult)
                    du = dp.tile([BL, H], bf16, tag="du")
                    nc.vector.tensor_reduce(
                        out=du[:], in_=slot[:, :, 0], axis=AX.X, op=OP.add)
                    nc.vector.tensor_tensor(
                        out=ubf[:], in0=ubf[:], in1=du[:], op=OP.add)
                    # y-half mult (off the chain) on idle GPSIMD
                    nc.gpsimd.tensor_tensor(
                        out=slot[:, :, 1],
                        in0=kH[:, m, :, 1],
                        in1=dbf[:].rearrange("p (o c) -> p o c", o=1)
                        .to_broadcast([BL, H, C]),
                        op=OP.mult)
                    dprev = dbf
                    Pcur = Pnxt
                    if nxt_gen is not None:
                        try:
                            next(nxt_gen)()
                        except StopIteration:
                            nxt_gen = None
                    if n % YF == YF - 1:
                        yr = yp.tile([BL, H], f32, tag="yr")
                        nc.vector.tensor_reduce(
                            out=yr[:], in_=yms[:, :, :, 1],
                            axis=AX.XY, op=OP.add)
                        nc.vector.tensor_tensor(
                            out=y[:], in0=y[:], in1=yr[:], op=OP.add)
                while nxt_gen is not None:
                    try:
                        next(nxt_gen)()
                    except StopIteration:
                        nxt_gen = None
                cur = nxt

            # ---- readout: out = (y @ rw + rb) @ ow + ob, transposed ----
            yT_ps = psum_r.tile([H, BL], f32, tag="yT")
            nc.tensor.transpose(out=yT_ps[:], in_=y[:], identity=ident[:])
            yT = yp.tile([H, BL], f32, tag="yT_sb")
            nc.scalar.copy(out=yT[:], in_=yT_ps[:])
            r1_ps = psum_r.tile([H, BL], f32, tag="r1")
            nc.tensor.matmul(out=r1_ps[:], lhsT=rw_sb[:], rhs=yT[:],
                             start=True, stop=True)
            r1 = yp.tile([H, BL], f32, tag="r1_sb")
            nc.scalar.add(out=r1[:], in_=r1_ps[:], add=rb_sb[:])
            o_ps = psum_r.tile([V, BL], f32, tag="o")
            nc.tensor.matmul(out=o_ps[:], lhsT=ow_sb[:], rhs=r1[:],
                             start=True, stop=True)
            o_sb = yp.tile([V, BL], f32, tag="o_sb")
            nc.scalar.add(out=o_sb[:], in_=o_ps[:], add=ob_sb[:])
            nc.sync.dma_start(outT.ap(), o_sb[:])

    nc.compile()
    return nc


def _host_tables(embed, w1, b1, w2, b2, ln_g, ln_b):
    f = np.float32
    h = embed.astype(f)
    ff = np.maximum(h @ w1.astype(f) + b1.astype(f), f(0)) @ w2.astype(f) \
        + b2.astype(f)
    x = h + ff
    mu = x.mean(-1, keepdims=True, dtype=f)
    var = ((x - mu) ** 2).mean(-1, keepdims=True, dtype=f)
    lut = ((x - mu) / np.sqrt(var + f(LN_EPS)) * ln_g.astype(f)
           + ln_b.astype(f)).astype(f)
    alpha = f(1.0) / ((lut * lut).sum(-1) + f(DELTA_EPS))
    return lut, alpha


def kernel(seq, embed, w1, b1, w2, b2, ln_g, ln_b, read_w, read_b,
           out_w, out_b):
    import ml_dtypes
    from concourse.bass_utils import run_bass_kernel_spmd
    bf = ml_dtypes.bfloat16
    f = np.float32

    seq = np.asarray(seq)
    lut, alpha = _host_tables(np.asarray(embed), np.asarray(w1),
                              np.asarray(b1), np.asarray(w2), np.asarray(b2),
                              np.asarray(ln_g), np.asarray(ln_b))
    G = (lut @ lut.T).astype(f)
    lut65 = np.concatenate([lut, np.zeros((1, H), f)], 0)
    alpha65 = np.concatenate([alpha, np.ones((1,), f)], 0)
    G65 = np.zeros((65, 65), f)
    G65[:64, :64] = G
    Gt2 = (-alpha65[:, None] * G65).astype(f)     # row-scaled gram

    tok = seq[:, L - 2::-1].astype(np.int64)      # [B, T] reversed
    tp = np.full((B, TP), 64, np.int64)
    tp[:, :T] = tok
    tpc = tp.reshape(B, NCH, C)
    q_all = lut[seq[:, L - 1]].astype(f)

    # host-gathered tensors (pure table lookups)
    K_i = lut65[tpc].astype(bf)                              # [B,NCH,C,H]
    # kHa: [B, NCH, H, 2, C]: a=0 -> k (u-update), a=1 -> k'' = -k/alpha (y)
    kpp65 = (-(1.0 / alpha65))[:, None] * lut65              # k'' table
    Kpp = kpp65[tpc].astype(bf)                              # [B,NCH,C,H]
    K_h = np.ascontiguousarray(
        np.stack([np.swapaxes(K_i, 2, 3), np.swapaxes(Kpp, 2, 3)],
                 axis=3))                                    # [B,NCH,H,2,C]
    ab = Gt2[tpc[..., :, None], tpc[..., None, :]]           # [B,NCH,C,C]
    il = np.tril(np.ones((C, C), bool), -1)
    abar_np = np.where(il, ab, 0.0)
    dg = (-alpha65[tpc]).astype(f)
    for e in range(C):
        abar_np[:, :, e, e] = dg[:, :, e]
    abar_np = abar_np.astype(bf)
    gcr_np = np.zeros((B, NCH, C, C), np.float32)
    gcr_np[:, 1:] = G65[tpc[:, 1:, :, None], tpc[:, :-1, None, :]]
    gcr_np = gcr_np.astype(bf)
    x0_np = np.zeros((B, NCH, C, C), np.float32)
    for e in range(C):
        x0_np[:, :, e, e] = dg[:, :, e]
    x0_np = x0_np.astype(bf)


    rw_np = np.asarray(read_w, f)
    rb_np = np.asarray(read_b, f).reshape(H, 1)
    ow_np = np.asarray(out_w, f)
    ob_np = np.asarray(out_b, f).reshape(V, 1)

    if "nc" not in _BUILT:
        _BUILT["nc"] = _build_module()
    nc = _BUILT["nc"]

    in_maps = []
    for cix in range(N_CORES):
        sl = slice(cix * BL, (cix + 1) * BL)
        in_maps.append({
            "ktI": np.ascontiguousarray(K_i[sl]),
            "ktH": np.ascontiguousarray(K_h[sl]),
            "abar": np.ascontiguousarray(abar_np[sl]),
            "x0": np.ascontiguousarray(x0_np[sl]),
            "gcr": np.ascontiguousarray(gcr_np[sl]),
            "qin": np.ascontiguousarray(q_all[sl]),
            "rw": rw_np, "rb": rb_np, "ow": ow_np, "ob": ob_np,
        })

    import os
    trace = os.environ.get("KERNEL_TRACE", "0") == "1"
    res = run_bass_kernel_spmd(nc, in_maps, core_ids=list(range(N_CORES)),
                               trace=trace)
    _BUILT["last_result"] = res
    out = np.empty((B, V), f)
    for cix in range(N_CORES):
        out[cix * BL:(cix + 1) * BL] = res.results[cix]["outT"].T
    return out
